# revision 1
# baseline (speedup 1.0000x reference)
"""Classwise-ECE (segmentation) kernel for 8 Trainium2 NeuronCores.

Math: with conf = softmax(logits, axis=C) laid out [C, N], bins
b = ceil(15*conf)-1 in [0,15), the reference ECE reduces to
    sce = mean_c sum_b |D[c,b]| / N,
    D[c,b] = sum_n (conf[c,n] - 1[label_n==c]) * 1[bin(conf[c,n])==b]
because |avg_conf-acc|*count == |conf_sum - acc_sum| per (c,b) bucket.

Sharding: pixels (N = B*H*W) split across 8 cores. Each core computes
partial D histograms [19,15]; host sums them and finalizes the scalar.

Per-core layout: partitions = 6 pixel-slots x 19 classes = 114 rows,
pixels on the free axis. Per 512-pixel chunk:
  exp on ACT (f32r out, feeding the tensor engine at 1 cyc/row);
  per-slot softmax denominators S via block-ones f32r matmuls
  accumulated into a [70,512] PSUM tile (3 chunks at 32-row offsets,
  the only PSUM bases compute engines can address); one DVE reciprocal
  per 3-chunk group; 1/S broadcast back to all 19 class rows via a
  second block-ones matmul; conf = exp*bcast(1/S) on DVE;
  v = labeq - conf (labeq precomputed host-side, bf16) with a free
  accum_out giving the per-row total sum(v) (bin 0 is derived from it
  on the host); bin index via the round-to-int magic-bias trick on ACT
  (Copy then Relu); then 14 fused compare-multiply-accumulate
  (scalar_tensor_tensor) passes on DVE, one per bin 1..14,
  accumulating per-(slot,class)-row sums into an SBUF accumulator.
  Note: scalar_tensor_tensor is illegal on GpSimd (Pool opcode check)
  and supports no DVE 2x/4x perf modes, so the 14 bin passes at 1x
  dominate (~97% DVE busy).
"""

import numpy as np

C = 19
NB = 15
SLOTS = 6
P = SLOTS * C            # 114 partitions
FD = 512                 # pixels per chunk per slot
B, H, W = 4, 512, 1024
N = B * H * W            # 2097152 pixels
N_CORES = 8
NPC = N // N_CORES       # 262144 pixels per core
CHUNKS = -(-NPC // (SLOTS * FD))   # 86
NF = CHUNKS * FD         # 44032 pixels per slot
NPIX = SLOTS * NF        # 264192 incl. padding
NPAD = NPIX - NPC        # 2048 zero-logit pad pixels per core
GROUP = 3                # chunks per S-pack PSUM tile (32-row spacing)
SROWS = 32 * (GROUP - 1) + SLOTS   # 70 packed S partitions per group
RGROUPS = 7              # S-pack groups per phase-A/phase-B batch
MAGIC = 8388608.0        # 2^23

_CACHE = {}


def _build_program():
    from contextlib import ExitStack
    import concourse.bass as bass
    import concourse.tile as tile
    from concourse import bacc, mybir
    from concourse.ap import AP

    f32 = mybir.dt.float32
    f32r = mybir.dt.float32r
    bf16 = mybir.dt.bfloat16
    ALU = mybir.AluOpType
    ACTF = mybir.ActivationFunctionType

    nc = bacc.Bacc("TRN2", target_bir_lowering=False, debug=False,
                   num_devices=N_CORES)

    lg = nc.dram_tensor("lg", [P, NF], f32, kind="ExternalInput").ap()
    le = nc.dram_tensor("le", [P, NF], bf16, kind="ExternalInput").ap()
    w1 = nc.dram_tensor("w1", [P, GROUP * SROWS], f32r,
                        kind="ExternalInput").ap()
    w2 = nc.dram_tensor("w2", [SROWS, P], f32, kind="ExternalInput").ap()
    hist = nc.dram_tensor("hist", [P, NB], f32, kind="ExternalOutput").ap()

    def block_ap(tile_ap, fd):
        # rows {0..5, 32..37, 64..69} of a [SROWS, fd] tile as [3, 6, fd]
        return AP(tile_ap.tensor, tile_ap.offset,
                  [[32, GROUP], [1, SLOTS]] + [list(p) for p in
                                               tile_ap.ap[1:]])

    with tile.TileContext(nc) as tc, ExitStack() as ctx:
        const_pool = ctx.enter_context(tc.tile_pool(name="const", bufs=1))
        in_pool = ctx.enter_context(tc.tile_pool(name="inp", bufs=4))
        le_pool = ctx.enter_context(tc.tile_pool(name="lep", bufs=26))
        et_pool = ctx.enter_context(tc.tile_pool(name="et", bufs=26))
        wk_pool = ctx.enter_context(tc.tile_pool(name="wk", bufs=3))
        r_pool = ctx.enter_context(tc.tile_pool(name="rp", bufs=8))
        sc_pool = ctx.enter_context(tc.tile_pool(name="sc", bufs=2))
        ps_s = ctx.enter_context(
            tc.tile_pool(name="ps_s", bufs=3, space=bass.MemorySpace.PSUM))
        ps_rb = ctx.enter_context(
            tc.tile_pool(name="ps_rb", bufs=3, space=bass.MemorySpace.PSUM))

        w1_sb = const_pool.tile([P, GROUP * SROWS], f32r)
        nc.sync.dma_start(w1_sb[:], w1)
        w2_sb = const_pool.tile([SROWS, P], f32)
        nc.sync.dma_start(w2_sb[:], w2)
        negm = const_pool.tile([P, 1], f32)
        nc.gpsimd.memset(negm[:], -MAGIC)
        acc = const_pool.tile([P, NB * CHUNKS], f32)

        ngroups = -(-CHUNKS // GROUP)   # 29
        # phase A: per group of 3 chunks, load + exp + S matmuls,
        # then DMA-pack S dense; per RGROUPS groups one reciprocal.
        ets = {}
        les = {}
        rpacks = {}

        for rg in range(-(-ngroups // RGROUPS)):   # 5 reciprocal batches
            gs = list(range(rg * RGROUPS, min((rg + 1) * RGROUPS, ngroups)))
            for q, g in enumerate(gs):
                ks = list(range(g * GROUP, min((g + 1) * GROUP, CHUNKS)))
                spack = ps_s.tile([SROWS, FD], f32, tag="spack")
                for j, k in enumerate(ks):
                    lt = in_pool.tile([P, FD], f32, tag="lt")
                    nc.sync.dma_start(lt[:], lg[:, k * FD:(k + 1) * FD])
                    lej = le_pool.tile([P, FD], bf16, tag="le")
                    nc.sync.dma_start(lej[:], le[:, k * FD:(k + 1) * FD])
                    et = et_pool.tile([P, FD], f32r, tag="et")
                    nc.scalar.activation(et[:], lt[:], ACTF.Exp)
                    nc.tensor.matmul(
                        spack[:],
                        w1_sb[:, j * SROWS:(j + 1) * SROWS],
                        et[:],
                        start=(j == 0), stop=(j == len(ks) - 1))
                    ets[k] = et
                    les[k] = lej
                rpk = r_pool.tile([SROWS, FD], f32, tag="rpack")
                nc.vector.reciprocal(rpk[:], spack[:])
                rpacks[g] = rpk

            # phase B for the groups in this reciprocal batch
            for g in gs:
                ks = list(range(g * GROUP, min((g + 1) * GROUP, CHUNKS)))
                rpk = rpacks.pop(g)
                for j, k in enumerate(ks):
                    rb = ps_rb.tile([P, FD], f32, tag="rb")
                    nc.tensor.matmul(
                        rb[:],
                        w2_sb[32 * j:32 * j + SLOTS, :],
                        rpk[32 * j:32 * j + SLOTS, :],
                        start=True, stop=True)
                    et = ets.pop(k)
                    lej = les.pop(k)
                    cf = wk_pool.tile([P, FD], f32, tag="cf")
                    nc.vector.tensor_mul(cf[:], et[:].bitcast(f32), rb[:])
                    vt = wk_pool.tile([P, FD], bf16, tag="vt")
                    nc.vector.scalar_tensor_tensor(
                        vt[:], lej[:], 1.0, cf[:],
                        op0=ALU.mult, op1=ALU.subtract,
                        accum_out=acc[:, k:k + 1])
                    y = wk_pool.tile([P, FD], f32, tag="y")
                    nc.scalar.activation(y[:], cf[:], ACTF.Copy,
                                         bias=MAGIC - 0.5, scale=15.0)
                    bi = wk_pool.tile([P, FD], bf16, tag="bi")
                    nc.scalar.activation(bi[:], y[:], ACTF.Relu,
                                         bias=negm[:], scale=1.0)
                    trash = wk_pool.tile([P, FD], bf16, tag="trash")
                    for t in range(1, NB):
                        col = t * CHUNKS + k
                        nc.vector.scalar_tensor_tensor(
                            trash[:], bi[:], float(t), vt[:],
                            op0=ALU.is_equal, op1=ALU.mult,
                            accum_out=acc[:, col:col + 1])

        hist_sb = const_pool.tile([P, NB], f32)
        acc3 = acc[:].rearrange("p (t k) -> p t k", k=CHUNKS)
        nc.vector.tensor_reduce(hist_sb[:], acc3, axis=mybir.AxisListType.X,
                                op=mybir.AluOpType.add)
        nc.sync.dma_start(hist, hist_sb[:])

    nc.compile()
    return nc


def _get_program():
    if "nc" not in _CACHE:
        _CACHE["nc"] = _build_program()
    return _CACHE["nc"]


def _host_constants():
    w1 = np.zeros((P, GROUP * SROWS), np.float32)
    w2 = np.zeros((SROWS, P), np.float32)
    for s in range(SLOTS):
        for c in range(C):
            p = s * C + c
            for j in range(GROUP):
                w1[p, j * SROWS + 32 * j + s] = 1.0
                w2[32 * j + s, p] = 1.0
    return w1, w2


def kernel(logits, labels, _trace=False):
    import ml_dtypes
    from concourse.bass_utils import run_bass_kernel_spmd

    logits = np.asarray(logits, dtype=np.float32)
    labels = np.asarray(labels)
    lt = np.moveaxis(logits, 1, 0).reshape(C, N)
    lf = labels.reshape(N).astype(np.int32)

    w1, w2 = _host_constants()
    cids = np.arange(C, dtype=np.int32)
    in_maps = []
    for i in range(N_CORES):
        sl = slice(i * NPC, (i + 1) * NPC)
        lgc = np.zeros((C, NPIX), np.float32)
        lgc[:, :NPC] = lt[:, sl]
        lgc = np.ascontiguousarray(
            lgc.reshape(C, SLOTS, NF).transpose(1, 0, 2).reshape(P, NF))
        lbc = np.zeros((NPIX,), np.int32)
        lbc[:NPC] = lf[sl]
        # labeq[s*19+c, j] = (label of pixel (s, j) == c), bf16
        lec = (lbc.reshape(SLOTS, 1, NF) == cids[None, :, None])
        lec = np.ascontiguousarray(
            lec.reshape(P, NF).astype(ml_dtypes.bfloat16))
        in_maps.append({"lg": lgc, "le": lec, "w1": w1, "w2": w2})

    nc = _get_program()
    res = run_bass_kernel_spmd(nc, in_maps, list(range(N_CORES)),
                               trace=_trace)
    _CACHE["last_exec_ns"] = res.exec_time_ns

    hist_agg = np.zeros((P, NB), np.float64)
    for r in res.results:
        hist_agg += r["hist"].astype(np.float64)
    hist_cb = hist_agg.reshape(SLOTS, C, NB).sum(axis=0)   # [19, 15]
    # col 0 holds sum(v) over all bins; recover the bin-0 partial
    hist_cb[:, 0] = hist_cb[:, 0] - hist_cb[:, 1:].sum(axis=1)
    # remove zero-logit padding (label 0, conf 1/19 -> bin 0)
    pad_total = NPAD * N_CORES
    r19 = np.float64(np.float32(1.0) / np.float32(19.0))
    hist_cb[:, 0] -= pad_total * ((np.arange(C) == 0).astype(np.float64) - r19)
    D = -hist_cb
    sce = np.abs(D).sum(axis=1).mean() / N
    return np.float32(sce)



# revision 8
# speedup vs baseline: 1.0811x; 1.0811x over previous
"""Classwise-ECE (segmentation) kernel for 8 Trainium2 NeuronCores.

Composite-value histogram design. With conf = softmax(logits, axis=C)
laid out [C, N] and bins b = ceil(15*conf)-1, the ECE reduces to
    sce = mean_c sum_b |D[c,b]| / N,   D[c,b] = sum_n v * [bin == b],
    v = 1[label==c] - conf.
Encode each element as one fp16 composite
    z' = b + u,  u = 0.5 + v/4  (u in [0.25, 0.75]),
then extract everything with per-threshold accumulation passes
("functionals") that run at DVE 4x speed (tensor_scalar, 2-byte packed
SBUF operands) or on the ACT engine (activation + accum_out):
    RS(t) = sum relu(z' - t)            t = 0..14   (DVE: add/max TS)
    CC(s) = #{z' > s}                   s = 1..14
        DVE form: 4*(sum clamp(z',s,s+1/4) - n*s)   (max/min TS)
        ACT form: (sum Sign(z'-s) + n)/2
Decode on host (f64):  A(t) = RS(t)-RS(t+1)-CC(t+1),
    C(t) = CC(t)-CC(t+1), D(t) = 4*A(t) - 2*C(t).

Pixels split across 8 cores; per core layout [114 = 6 slots x 19
classes, 44032 pixels]. Softmax: exp on ACT (bf16), per-pixel sums via
block-routing matmuls packed 16 chunks deep ([96,512] PSUM -> one DVE
reciprocal per 16 chunks), 1/S broadcast back by a second routing
matmul, conf/4 = et * rb on DVE (fp16 out). Bin index via the fp16
round-to-int trick: y16 = fp16(60*cf4 + 1023.5) rounds to 1024 + b
exactly (fp16 quantum is 1.0 there); bih = (y16 - 1023.5) max 0.5.
"""

import numpy as np

C = 19
NB = 15
SLOTS = 6
P = SLOTS * C            # 114 partitions
FD = 512                 # pixels per softmax chunk
B, H, W = 4, 512, 1024
N = B * H * W            # 2097152 pixels
N_CORES = 8
NPC = N // N_CORES       # 262144 pixels per core
CHUNKS = -(-NPC // (SLOTS * FD))   # 86
NF = CHUNKS * FD         # 44032 pixels per slot-row
NPIX = SLOTS * NF        # 264192 incl. padding
NPAD = NPIX - NPC        # 2048 zero-logit pad pixels per core
G = 16                   # softmax chunks per S-pack / reciprocal group
NGROUPS = -(-CHUNKS // G)          # 6 (5 full + ragged 6)
HB = 8                   # softmax chunks per stage-2 big chunk
NKB = -(-CHUNKS // HB)   # 11 (10 full + ragged 6)
MAGIC16 = 1023.5         # fp16 round-to-int bias (quantum 1.0 at 1024)

# functional -> engine split (tune from traces)
DVE_RS = list(range(NB))            # RS(0..14) on DVE, add/max TS 4x
DVE_CC = [1, 2, 3]                  # clamp-counts on DVE
ACT_CC = list(range(4, NB))         # sign-counts on ACT
NFUNC = NB + (NB - 1)               # 29 accum families
# accum column layout: RS t -> row t; CC s -> row 15 + (s-1)
_RS_COL = {t: t for t in range(NB)}
_CC_COL = {s: NB + s - 1 for s in range(1, NB)}

_CACHE = {}


def _slices_of_group(g):
    return range(g * G, min((g + 1) * G, CHUNKS))


def _kbs_of_group(g):
    ks = _slices_of_group(g)
    return sorted({k // HB for k in ks})


def _slices_of_kb(kb):
    return range(kb * HB, min((kb + 1) * HB, CHUNKS))


def _build_program():
    from contextlib import ExitStack
    import concourse.bass as bass
    import concourse.tile as tile
    from concourse import bacc, mybir

    f32 = mybir.dt.float32
    f16 = mybir.dt.float16
    bf16 = mybir.dt.bfloat16
    ALU = mybir.AluOpType
    ACTF = mybir.ActivationFunctionType

    nc = bacc.Bacc("TRN2", target_bir_lowering=False, debug=False,
                   num_devices=N_CORES)

    lg = nc.dram_tensor("lg", [P, NF], f32, kind="ExternalInput").ap()
    le = nc.dram_tensor("le", [P, NF], f16, kind="ExternalInput").ap()
    w1 = nc.dram_tensor("w1", [P, G * SLOTS * G], bf16,
                        kind="ExternalInput").ap()
    w2 = nc.dram_tensor("w2", [G * SLOTS, G * P], f32,
                        kind="ExternalInput").ap()
    hacc = nc.dram_tensor("hacc", [P, NFUNC * NKB], f32,
                          kind="ExternalOutput").ap()

    SR = G * SLOTS           # 96 packed S rows per group

    with tile.TileContext(nc) as tc, ExitStack() as ctx:
        const_pool = ctx.enter_context(tc.tile_pool(name="const", bufs=1))
        lt_pool = ctx.enter_context(tc.tile_pool(name="lt", bufs=2))
        le_pool = ctx.enter_context(tc.tile_pool(name="le", bufs=2))
        et_pool = ctx.enter_context(tc.tile_pool(name="et", bufs=3))
        cf_pool = ctx.enter_context(tc.tile_pool(name="cf", bufs=2))
        vt_pool = ctx.enter_context(tc.tile_pool(name="vt", bufs=2))
        y_pool = ctx.enter_context(tc.tile_pool(name="y", bufs=1))
        bih_pool = ctx.enter_context(tc.tile_pool(name="bih", bufs=1))
        zp_pool = ctx.enter_context(tc.tile_pool(name="zp", bufs=2))
        td_pool = ctx.enter_context(tc.tile_pool(name="td", bufs=2))
        ta_pool = ctx.enter_context(tc.tile_pool(name="ta", bufs=2))
        rp_pool = ctx.enter_context(tc.tile_pool(name="rp", bufs=2))
        ps_s = ctx.enter_context(
            tc.tile_pool(name="ps_s", bufs=2, space=bass.MemorySpace.PSUM))
        ps_rb = ctx.enter_context(
            tc.tile_pool(name="ps_rb", bufs=4, space=bass.MemorySpace.PSUM))

        w1_sb = const_pool.tile([P, G * SR], bf16)
        nc.sync.dma_start(w1_sb[:], w1)
        w2_sb = const_pool.tile([SR, G * P], f32)
        nc.sync.dma_start(w2_sb[:], w2)
        acc = const_pool.tile([P, NFUNC * NKB], f32)
        sbias = const_pool.tile([P, len(ACT_CC)], f32)
        for i, s in enumerate(ACT_CC):
            nc.gpsimd.memset(sbias[:, i:i + 1], -float(s))

        lts = {}
        les = {}
        ets = {}

        for g in range(NGROUPS):
            ks = list(_slices_of_group(g))
            kbs = _kbs_of_group(g)
            # stage 1 prologue: load + exp per big chunk of this group
            for kb in kbs:
                if kb in ets:
                    continue
                fdb = len(list(_slices_of_kb(kb))) * FD
                off = kb * HB * FD
                ltb = lt_pool.tile([P, fdb], f32, tag="lt")
                nc.sync.dma_start(ltb[:], lg[:, off:off + fdb])
                leb = le_pool.tile([P, fdb], f16, tag="le")
                nc.sync.dma_start(leb[:], le[:, off:off + fdb])
                etb = et_pool.tile([P, fdb], bf16, tag="et")
                nc.scalar.activation(etb[:], ltb[:], ACTF.Exp)
                lts[kb] = ltb
                les[kb] = leb
                ets[kb] = etb

            # packed S matmuls: route chunk jg's slot sums to rows 6*jg+s
            srows = SLOTS * len(ks)
            spack = ps_s.tile([srows, FD], f32, tag="spack")
            for jg, k in enumerate(ks):
                kb, j = k // HB, k % HB
                etsl = ets[kb][:, j * FD:(j + 1) * FD]
                nc.tensor.matmul(
                    spack[:],
                    w1_sb[:, jg * SR:jg * SR + srows],
                    etsl,
                    start=(jg == 0), stop=(jg == len(ks) - 1))
            rpk = rp_pool.tile([srows, FD], f32, tag="rpk")
            nc.vector.reciprocal(rpk[:], spack[:])

            # stage 2 per big chunk
            for kb in kbs:
                ksl = [k for k in _slices_of_kb(kb) if k in ks]
                assert len(ksl) == len(list(_slices_of_kb(kb))), \
                    "group/big-chunk misalignment"
                fdb = len(ksl) * FD
                etb = ets[kb]
                cfb = cf_pool.tile([P, fdb], f16, tag="cf")
                for k in ksl:
                    jg, j = k - g * G, k % HB
                    rb = ps_rb.tile([P, FD], f32, tag="rb")
                    nc.tensor.matmul(
                        rb[:],
                        w2_sb[:srows, jg * P:(jg + 1) * P],
                        rpk[:],
                        start=True, stop=True)
                    nc.vector.tensor_mul(
                        cfb[:, j * FD:(j + 1) * FD],
                        etb[:, j * FD:(j + 1) * FD], rb[:])

                leb = les.pop(kb)
                lts.pop(kb)
                ets.pop(kb)
                vtb = vt_pool.tile([P, fdb], f16, tag="vt")
                nc.vector.tensor_sub(vtb[:], leb[:], cfb[:])
                y16 = y_pool.tile([P, fdb], f16, tag="y16")
                nc.vector.tensor_scalar(
                    y16[:], cfb[:], 60.0, MAGIC16, op0=ALU.mult, op1=ALU.add)
                bih = bih_pool.tile([P, fdb], f16, tag="bih")
                nc.vector.tensor_scalar(
                    bih[:], y16[:], -MAGIC16, 0.5, op0=ALU.add, op1=ALU.max)
                zpb = zp_pool.tile([P, fdb], f16, tag="zp")
                nc.vector.tensor_add(zpb[:], bih[:], vtb[:])

                # tensor_scalar with accum_out: out = in0 op0 scalar1,
                # accum_out = reduce(out, op1, init=scalar2)
                trd = td_pool.tile([P, fdb], f16, tag="trd")
                for t in DVE_RS:
                    col = _RS_COL[t] * NKB + kb
                    # sum max(z', t) = RS(t) + n*t
                    nc.vector.tensor_scalar(
                        trd[:], zpb[:], float(t), 0.0,
                        op0=ALU.max, op1=ALU.add,
                        accum_out=acc[:, col:col + 1])
                for s in DVE_CC:
                    col = _CC_COL[s] * NKB + kb
                    # sum [z' > s] = CC(s)
                    nc.vector.tensor_scalar(
                        trd[:], zpb[:], float(s), 0.0,
                        op0=ALU.is_gt, op1=ALU.add,
                        accum_out=acc[:, col:col + 1])
                tra = ta_pool.tile([P, fdb], f16, tag="tra")
                for i, s in enumerate(ACT_CC):
                    col = _CC_COL[s] * NKB + kb
                    nc.scalar.activation(
                        tra[:], zpb[:], ACTF.Sign,
                        bias=sbias[:, i:i + 1],
                        accum_out=acc[:, col:col + 1])

        nc.sync.dma_start(hacc, acc[:])

    nc.compile()
    return nc


def _get_program():
    if "nc" not in _CACHE:
        _CACHE["nc"] = _build_program()
    return _CACHE["nc"]


def _host_constants():
    import ml_dtypes
    SR = G * SLOTS
    w1 = np.zeros((P, G * SR), np.float32)
    w2 = np.zeros((SR, G * P), np.float32)
    for jg in range(G):
        for s in range(SLOTS):
            for c in range(C):
                p = s * C + c
                w1[p, jg * SR + SLOTS * jg + s] = 1.0
                w2[SLOTS * jg + s, jg * P + p] = 0.25
    return w1.astype(ml_dtypes.bfloat16), w2


def kernel(logits, labels, _trace=False):
    from concourse.bass_utils import run_bass_kernel_spmd

    logits = np.asarray(logits, dtype=np.float32)
    labels = np.asarray(labels)
    lt = np.moveaxis(logits, 1, 0).reshape(C, N)
    lf = labels.reshape(N).astype(np.int32)

    w1, w2 = _host_constants()
    cids = np.arange(C, dtype=np.int32)
    in_maps = []
    for i in range(N_CORES):
        sl = slice(i * NPC, (i + 1) * NPC)
        lgc = np.zeros((C, NPIX), np.float32)
        lgc[:, :NPC] = lt[:, sl]
        lgc = np.ascontiguousarray(
            lgc.reshape(C, SLOTS, NF).transpose(1, 0, 2).reshape(P, NF))
        lbc = np.zeros((NPIX,), np.int32)
        lbc[:NPC] = lf[sl]
        lec = (lbc.reshape(SLOTS, 1, NF) == cids[None, :, None])
        lec = np.ascontiguousarray(
            (lec.reshape(P, NF).astype(np.float32) * 0.25).astype(np.float16))
        in_maps.append({"lg": lgc, "le": lec, "w1": w1, "w2": w2})

    nc = _get_program()
    res = run_bass_kernel_spmd(nc, in_maps, list(range(N_CORES)),
                               trace=_trace)
    _CACHE["last_exec_ns"] = res.exec_time_ns

    hsum = np.zeros((P, NFUNC * NKB), np.float64)
    for r in res.results:
        hsum += r["hacc"].astype(np.float64)
    h = hsum.reshape(P, NFUNC, NKB)

    NT = N_CORES * NF  # elements per row across cores
    RS = np.zeros((P, NB + 1))
    CCm = np.zeros((P, NB + 1))
    for t in range(NB):
        # DVE form: sum max(z', t) = RS(t) + n*t
        RS[:, t] = h[:, _RS_COL[t], :].sum(axis=1) - NT * t
    CCm[:, 0] = NT
    for s in range(1, NB):
        val = h[:, _CC_COL[s], :].sum(axis=1)
        if s in DVE_CC:
            CCm[:, s] = val                    # sum [z' > s]
        else:
            CCm[:, s] = (val + NT) / 2.0       # Sign decode

    A = RS[:, :NB] - RS[:, 1:] - CCm[:, 1:]
    Cn = CCm[:, :NB] - CCm[:, 1:]
    D = 4.0 * A - 2.0 * Cn                     # [P, NB], sum of v per bin
    D_cb = D.reshape(SLOTS, C, NB).sum(axis=0)  # [19, 15]

    # remove zero-logit padding (label 0, conf 1/19 -> bin 0)
    pad_total = NPAD * N_CORES
    r19 = np.float64(np.float32(1.0) / np.float32(19.0))
    D_cb[:, 0] -= pad_total * ((np.arange(C) == 0).astype(np.float64) - r19)

    sce = np.abs(D_cb).sum(axis=1).mean() / N
    return np.float32(sce)


# revision 13
# speedup vs baseline: 1.9393x; 1.7937x over previous
"""Classwise-ECE (segmentation) kernel for 8 Trainium2 NeuronCores.

Hybrid histogram design. With conf = softmax(logits, axis=C) laid out
[C, N] and bins b = ceil(15*conf)-1, the ECE reduces to
    sce = mean_c sum_b |D[c,b]| / N,   D[c,b] = sum_n v * [bin == b],
    v = 1[label==c] - conf.

Measured engine facts driving the design (TRN2):
  - DVE tensor_scalar WITHOUT accum_out runs at 4x with fp16 packed
    SBUF operands (~0.29 ns/elem); WITH accum_out it lowers to
    TENSOR_SCALAR_CACHE_REDUCE at 1x (~1.08 ns/elem).
  - ACT activation supports accum_out at ~0.95 ns/elem (Relu/Sign).
  - GpSimd tensor_scalar + accum_out is legal (is_gt counts, exact).
  - scalar_tensor_tensor (stt, 1x, DVE-only) does (in0 op0 s) op1 in1
    with sum-accum: a direct masked D-sum in ONE pass.

Per element, fp16 intermediates (validated ~2e-4 end-to-end on host):
  et  = exp(logit)              bf16 (ACT)
  S   = packed routing matmuls -> [96,512] PSUM per 16 chunks (PE)
  rpk = 1/S                     bf16 (one DVE reciprocal per 16 chunks)
  cf4 = et * bcast(0.25/S)      fp16 (DVE 1x over 4-bank PSUM regions)
  y16 = fp16(60*cf4 + 1023.5)   == 1024 + b exactly (DVE TS 4x)
  bih = (y16 - 1023.5) max 0.5  == b + 0.5 (DVE TS 4x)
  vt4 = lej4 - cf4 (stt, accum -> sum v/4)  == v/4
  zp  = bih + vt4               (DVE TT 2x) == b + 0.5 + v/4

Bins 1..T0-1 ("low"): direct stt functionals on DVE (bih is bf16 so
the stt reads one bf16 + one fp16 source -- dual non-bf16 sources
would halve DVE throughput):
    D(b)/4 = sum [bih == b+0.5] * vt4.
Bins T0..14 ("high"): composite decode on ACT (Pool/GpSimd rejects
accum opcodes at the ISA level, so only ACT can offload these):
    RS(t) = sum relu(zp - t)       (Relu + accum)
    CC(s) = #{zp > s}              (Sign + accum, decode (val+n)/2)
    A(t) = RS(t)-RS(t+1)-CC(t+1); C(t) = CC(t)-CC(t+1)
    D(t) = 4*A(t) - 2*C(t)
Bin 0: D(0) = 4*sum(vt4) - sum_{b>=1} D(b).
"""

import numpy as np

C = 19
NB = 15
SLOTS = 6
P = SLOTS * C            # 114 partitions
FD = 512                 # pixels per softmax chunk
B, H, W = 4, 512, 1024
N = B * H * W            # 2097152 pixels
N_CORES = 8
NPC = N // N_CORES       # 262144 pixels per core
CHUNKS = -(-NPC // (SLOTS * FD))   # 86
NF = CHUNKS * FD         # 44032 pixels per slot-row
NPIX = SLOTS * NF        # 264192 incl. padding
NPAD = NPIX - NPC        # 2048 zero-logit pad pixels per core
G = 16                   # softmax chunks per S-pack / reciprocal group
NGROUPS = -(-CHUNKS // G)          # 6 (5 full + ragged 6)
HB = 8                   # softmax chunks per stage-2 big chunk
NKB = -(-CHUNKS // HB)   # 11 (10 full + ragged 6)
RBW = 4                  # softmax chunks per cf4 batch (4-bank PSUM region)
MAGIC16 = 1023.5         # fp16 round-to-int bias (quantum 1.0 at 1024)

T0 = 9                   # bins 1..T0-1 via stt; T0..14 via composite
STT_BINS = list(range(1, T0))            # 5 DVE stt functionals
RS_TS = list(range(T0, NB))              # RS(6..14), 9 functionals
CC_TS = list(range(T0, NB))              # CC(6..14), 9 functionals
GPS_CC = []                              # Pool rejects accum opcodes
ACT_CC = CC_TS                           # Sign counts on ACT

# accum column layout in `acc` [P, NFUNC, NKB]:
#   row 0:            sum vt4 (from the vt build stt)
#   rows 1..T0-1:     stt bins
#   next len(RS_TS):  RS
#   next len(CC_TS):  CC
_VT_ROW = 0
_STT_ROW = {b: b for b in STT_BINS}
_RS_ROW = {t: T0 + i for i, t in enumerate(RS_TS)}
_CC_ROW = {s: T0 + len(RS_TS) + i for i, s in enumerate(CC_TS)}
NFUNC = T0 + len(RS_TS) + len(CC_TS)     # 24

_CACHE = {}


def _slices_of_group(g):
    return range(g * G, min((g + 1) * G, CHUNKS))


def _kbs_of_group(g):
    return sorted({k // HB for k in _slices_of_group(g)})


def _slices_of_kb(kb):
    return range(kb * HB, min((kb + 1) * HB, CHUNKS))


def _build_program():
    from contextlib import ExitStack
    import concourse.bass as bass
    import concourse.tile as tile
    from concourse import bacc, mybir

    f32 = mybir.dt.float32
    f16 = mybir.dt.float16
    bf16 = mybir.dt.bfloat16
    ALU = mybir.AluOpType
    ACTF = mybir.ActivationFunctionType

    nc = bacc.Bacc("TRN2", target_bir_lowering=False, debug=False,
                   num_devices=N_CORES)

    lg = nc.dram_tensor("lg", [P, NF], bf16, kind="ExternalInput").ap()
    le = nc.dram_tensor("le", [P, NF], bf16, kind="ExternalInput").ap()
    w1 = nc.dram_tensor("w1", [P, G * SLOTS * G], bf16,
                        kind="ExternalInput").ap()
    w2 = nc.dram_tensor("w2", [G * SLOTS, G * P], bf16,
                        kind="ExternalInput").ap()
    hacc = nc.dram_tensor("hacc", [P, NFUNC * NKB], f32,
                          kind="ExternalOutput").ap()

    SR = G * SLOTS           # 96 packed S rows per group

    with tile.TileContext(nc) as tc, ExitStack() as ctx:
        const_pool = ctx.enter_context(tc.tile_pool(name="const", bufs=1))
        lt_pool = ctx.enter_context(tc.tile_pool(name="lt", bufs=3))
        le_pool = ctx.enter_context(tc.tile_pool(name="le", bufs=2))
        et_pool = ctx.enter_context(tc.tile_pool(name="et", bufs=3))
        cf_pool = ctx.enter_context(tc.tile_pool(name="cf", bufs=2))
        vt_pool = ctx.enter_context(tc.tile_pool(name="vt", bufs=2))
        y_pool = ctx.enter_context(tc.tile_pool(name="y", bufs=1))
        bih_pool = ctx.enter_context(tc.tile_pool(name="bih", bufs=2))
        zp_pool = ctx.enter_context(tc.tile_pool(name="zp", bufs=2))
        td_pool = ctx.enter_context(tc.tile_pool(name="td", bufs=2))
        ta_pool = ctx.enter_context(tc.tile_pool(name="ta", bufs=2))
        rp_pool = ctx.enter_context(tc.tile_pool(name="rp", bufs=2))
        ps_s = ctx.enter_context(
            tc.tile_pool(name="ps_s", bufs=2, space=bass.MemorySpace.PSUM))
        ps_rb = ctx.enter_context(
            tc.tile_pool(name="ps_rb", bufs=1, space=bass.MemorySpace.PSUM))

        w1_sb = const_pool.tile([P, G * SR], bf16)
        nc.sync.dma_start(w1_sb[:], w1)
        w2_sb = const_pool.tile([SR, G * P], bf16)
        nc.sync.dma_start(w2_sb[:], w2)
        acc = const_pool.tile([P, NFUNC * NKB], f32)
        sbias = const_pool.tile([P, max(1, len(ACT_CC))], f32)
        for i, s in enumerate(ACT_CC):
            nc.gpsimd.memset(sbias[:, i:i + 1], -float(s))
        rbias = const_pool.tile([P, len(RS_TS)], f32)
        for i, t in enumerate(RS_TS):
            nc.gpsimd.memset(rbias[:, i:i + 1], -float(t))

        lts = {}
        les = {}
        ets = {}

        for g in range(NGROUPS):
            ks = list(_slices_of_group(g))
            kbs = _kbs_of_group(g)
            for kb in kbs:
                if kb in ets:
                    continue
                fdb = len(list(_slices_of_kb(kb))) * FD
                off = kb * HB * FD
                ltb = lt_pool.tile([P, fdb], bf16, tag="lt")
                nc.sync.dma_start(ltb[:], lg[:, off:off + fdb])
                leb = le_pool.tile([P, fdb], bf16, tag="le")
                nc.sync.dma_start(leb[:], le[:, off:off + fdb])
                etb = et_pool.tile([P, fdb], bf16, tag="et")
                nc.scalar.activation(etb[:], ltb[:], ACTF.Exp)
                lts[kb] = ltb
                les[kb] = leb
                ets[kb] = etb

            srows = SLOTS * len(ks)
            spack = ps_s.tile([srows, FD], f32, tag="spack")
            for jg, k in enumerate(ks):
                kb, j = k // HB, k % HB
                etsl = ets[kb][:, j * FD:(j + 1) * FD]
                nc.tensor.matmul(
                    spack[:],
                    w1_sb[:, jg * SR:jg * SR + srows],
                    etsl,
                    start=(jg == 0), stop=(jg == len(ks) - 1))
            rpk = rp_pool.tile([srows, FD], bf16, tag="rpk")
            with nc.allow_low_precision(reason="1/S to bf16: validated 1e-4"):
                nc.vector.reciprocal(rpk[:], spack[:])

            for kb in kbs:
                ksl = [k for k in _slices_of_kb(kb) if k in ks]
                assert len(ksl) == len(list(_slices_of_kb(kb))), \
                    "group/big-chunk misalignment"
                fdb = len(ksl) * FD
                etb = ets[kb]
                cfb = cf_pool.tile([P, fdb], f16, tag="cf")
                # rb batches of RBW chunks -> one wide cf4 multiply each
                for r0 in range(0, len(ksl), RBW):
                    rk = ksl[r0:r0 + RBW]
                    rbw = ps_rb.tile([P, len(rk) * FD], f32, tag="rb")
                    for q, k in enumerate(rk):
                        jg = k - g * G
                        nc.tensor.matmul(
                            rbw[:, q * FD:(q + 1) * FD],
                            w2_sb[:srows, jg * P:(jg + 1) * P],
                            rpk[:],
                            start=True, stop=True)
                    j0 = rk[0] % HB
                    nc.vector.tensor_mul(
                        cfb[:, j0 * FD:(j0 + len(rk)) * FD],
                        etb[:, j0 * FD:(j0 + len(rk)) * FD], rbw[:])

                leb = les.pop(kb)
                lts.pop(kb)
                ets.pop(kb)
                y16 = y_pool.tile([P, fdb], f16, tag="y16")
                nc.vector.tensor_scalar(
                    y16[:], cfb[:], 60.0, MAGIC16, op0=ALU.mult, op1=ALU.add)
                bih = bih_pool.tile([P, fdb], bf16, tag="bih")
                nc.vector.tensor_scalar(
                    bih[:], y16[:], -MAGIC16, 0.5, op0=ALU.add, op1=ALU.max)
                vtb = vt_pool.tile([P, fdb], f16, tag="vt")
                nc.vector.scalar_tensor_tensor(
                    vtb[:], leb[:], 1.0, cfb[:],
                    op0=ALU.mult, op1=ALU.subtract,
                    accum_out=acc[:, _VT_ROW * NKB + kb:_VT_ROW * NKB + kb + 1])
                zpb = zp_pool.tile([P, fdb], f16, tag="zp")
                nc.vector.tensor_add(zpb[:], bih[:], vtb[:])

                # low bins: direct masked D sums on DVE (stt, 1x)
                trd = td_pool.tile([P, fdb], f16, tag="trd")
                for b in STT_BINS:
                    col = _STT_ROW[b] * NKB + kb
                    nc.vector.scalar_tensor_tensor(
                        trd[:], bih[:], float(b) + 0.5, vtb[:],
                        op0=ALU.is_equal, op1=ALU.mult,
                        accum_out=acc[:, col:col + 1])
                # high bins: composite functionals on ACT + GpSimd
                tra = ta_pool.tile([P, fdb], f16, tag="tra")
                for i, t in enumerate(RS_TS):
                    col = _RS_ROW[t] * NKB + kb
                    nc.scalar.activation(
                        tra[:], zpb[:], ACTF.Relu,
                        bias=rbias[:, i:i + 1],
                        accum_out=acc[:, col:col + 1])
                for i, s in enumerate(ACT_CC):
                    col = _CC_ROW[s] * NKB + kb
                    nc.scalar.activation(
                        tra[:], zpb[:], ACTF.Sign,
                        bias=sbias[:, i:i + 1],
                        accum_out=acc[:, col:col + 1])

        nc.sync.dma_start(hacc, acc[:])

    nc.compile()
    return nc


def _get_program():
    if "nc" not in _CACHE:
        _CACHE["nc"] = _build_program()
    return _CACHE["nc"]


def _host_constants():
    import ml_dtypes
    SR = G * SLOTS
    w1 = np.zeros((P, G * SR), np.float32)
    w2 = np.zeros((SR, G * P), np.float32)
    for jg in range(G):
        for s in range(SLOTS):
            for c in range(C):
                p = s * C + c
                w1[p, jg * SR + SLOTS * jg + s] = 1.0
                w2[SLOTS * jg + s, jg * P + p] = 0.25
    return w1.astype(ml_dtypes.bfloat16), w2.astype(ml_dtypes.bfloat16)


def _decode(hsum, ncores=N_CORES):
    """hsum: [P, NFUNC*NKB] f64 (summed over cores) -> D_cb [19, 15]."""
    h = hsum.reshape(P, NFUNC, NKB)
    NT = ncores * NF
    sum_v4 = h[:, _VT_ROW, :].sum(axis=1)

    RS = np.zeros((P, NB + 1))
    CCm = np.zeros((P, NB + 1))
    for t in RS_TS:
        RS[:, t] = h[:, _RS_ROW[t], :].sum(axis=1)
    for s in CC_TS:
        val = h[:, _CC_ROW[s], :].sum(axis=1)
        if s in GPS_CC:
            CCm[:, s] = val
        else:
            CCm[:, s] = (val + NT) / 2.0

    D = np.zeros((P, NB))
    for b in STT_BINS:
        D[:, b] = 4.0 * h[:, _STT_ROW[b], :].sum(axis=1)
    for t in RS_TS:
        A = RS[:, t] - RS[:, t + 1] - CCm[:, t + 1]
        Cn = CCm[:, t] - CCm[:, t + 1]
        D[:, t] = 4.0 * A - 2.0 * Cn
    D[:, 0] = 4.0 * sum_v4 - D[:, 1:].sum(axis=1)

    return D.reshape(SLOTS, C, NB).sum(axis=0)


def kernel(logits, labels, _trace=False):
    import ml_dtypes
    from concourse.bass_utils import run_bass_kernel_spmd

    logits = np.asarray(logits, dtype=np.float32)
    labels = np.asarray(labels)
    lt = np.moveaxis(logits, 1, 0).reshape(C, N)
    lf = labels.reshape(N).astype(np.int32)

    w1, w2 = _host_constants()
    cids = np.arange(C, dtype=np.int32)
    in_maps = []
    for i in range(N_CORES):
        sl = slice(i * NPC, (i + 1) * NPC)
        lgc = np.zeros((C, NPIX), np.float32)
        lgc[:, :NPC] = lt[:, sl]
        lgc = np.ascontiguousarray(
            lgc.reshape(C, SLOTS, NF).transpose(1, 0, 2).reshape(P, NF)
        ).astype(ml_dtypes.bfloat16)
        lbc = np.zeros((NPIX,), np.int32)
        lbc[:NPC] = lf[sl]
        lec = (lbc.reshape(SLOTS, 1, NF) == cids[None, :, None])
        lec = np.ascontiguousarray(
            (lec.reshape(P, NF).astype(np.float32) * 0.25).astype(ml_dtypes.bfloat16))
        in_maps.append({"lg": lgc, "le": lec, "w1": w1, "w2": w2})

    nc = _get_program()
    res = run_bass_kernel_spmd(nc, in_maps, list(range(N_CORES)),
                               trace=_trace)
    _CACHE["last_exec_ns"] = res.exec_time_ns

    hsum = np.zeros((P, NFUNC * NKB), np.float64)
    for r in res.results:
        hsum += r["hacc"].astype(np.float64)
    D_cb = _decode(hsum)

    # remove zero-logit padding (label 0, conf 1/19 -> bin 0)
    pad_total = NPAD * N_CORES
    r19 = np.float64(np.float32(1.0) / np.float32(19.0))
    D_cb[:, 0] -= pad_total * ((np.arange(C) == 0).astype(np.float64) - r19)

    sce = np.abs(D_cb).sum(axis=1).mean() / N
    return np.float32(sce)


# revision 15
# speedup vs baseline: 1.9639x; 1.0127x over previous
"""Classwise-ECE (segmentation) kernel for 8 Trainium2 NeuronCores.

Hybrid histogram design. With conf = softmax(logits, axis=C) laid out
[C, N] and bins b = ceil(15*conf)-1, the ECE reduces to
    sce = mean_c sum_b |D[c,b]| / N,   D[c,b] = sum_n v * [bin == b],
    v = 1[label==c] - conf.

Measured engine facts driving the design (TRN2):
  - DVE tensor_scalar WITHOUT accum_out runs at 4x with fp16 packed
    SBUF operands (~0.29 ns/elem); WITH accum_out it lowers to
    TENSOR_SCALAR_CACHE_REDUCE at 1x (~1.08 ns/elem).
  - ACT activation supports accum_out at ~0.95 ns/elem (Relu/Sign).
  - GpSimd tensor_scalar + accum_out is legal (is_gt counts, exact).
  - scalar_tensor_tensor (stt, 1x, DVE-only) does (in0 op0 s) op1 in1
    with sum-accum: a direct masked D-sum in ONE pass.

Per element, fp16 intermediates (validated ~2e-4 end-to-end on host):
  et  = exp(logit)              bf16 (ACT)
  S   = packed routing matmuls -> [96,512] PSUM per 16 chunks (PE)
  rpk = 1/S                     bf16 (one DVE reciprocal per 16 chunks)
  cf4 = et * bcast(0.25/S)      fp16 (DVE 1x over 4-bank PSUM regions)
  y16 = fp16(60*cf4 + 1023.5)   == 1024 + b exactly (DVE TS 4x)
  bih = (y16 - 1023.5) max 0.5  == b + 0.5 (DVE TS 4x)
  vt4 = lej4 - cf4 (stt, accum -> sum v/4)  == v/4
  zp  = bih + vt4               (DVE TT 2x) == b + 0.5 + v/4

Bins 1..T0-1 ("low"): direct stt functionals on DVE (bih is bf16 so
the stt reads one bf16 + one fp16 source -- dual non-bf16 sources
would halve DVE throughput):
    D(b)/4 = sum [bih == b+0.5] * vt4.
Bins T0..14 ("high"): composite decode on ACT (Pool/GpSimd rejects
accum opcodes at the ISA level, so only ACT can offload these):
    RS(t) = sum relu(zp - t)       (Relu + accum)
    CC(s) = #{zp > s}              (Sign + accum, decode (val+n)/2)
    A(t) = RS(t)-RS(t+1)-CC(t+1); C(t) = CC(t)-CC(t+1)
    D(t) = 4*A(t) - 2*C(t)
Bin 0: D(0) = 4*sum(vt4) - sum_{b>=1} D(b).
"""

import numpy as np

C = 19
NB = 15
SLOTS = 6
P = SLOTS * C            # 114 partitions
FD = 512                 # pixels per softmax chunk
B, H, W = 4, 512, 1024
N = B * H * W            # 2097152 pixels
N_CORES = 8
NPC = N // N_CORES       # 262144 pixels per core
CHUNKS = -(-NPC // (SLOTS * FD))   # 86
NF = CHUNKS * FD         # 44032 pixels per slot-row
NPIX = SLOTS * NF        # 264192 incl. padding
NPAD = NPIX - NPC        # 2048 zero-logit pad pixels per core
G = 16                   # softmax chunks per S-pack / reciprocal group
NGROUPS = None                     # set below from group starts
HB = 8                   # softmax chunks per stage-2 big chunk
NKB = -(-CHUNKS // HB)   # 11 (10 full + ragged 6)
RBW = 4                  # softmax chunks per cf4 batch (4-bank PSUM region)
MAGIC16 = 1023.5         # fp16 round-to-int bias (quantum 1.0 at 1024)

T0 = 9                   # bins 1..T0-1 via stt; T0..14 via composite
STT_BINS = list(range(1, T0))            # 5 DVE stt functionals
RS_TS = list(range(T0, NB))              # RS(6..14), 9 functionals
CC_TS = list(range(T0, NB))              # CC(6..14), 9 functionals
GPS_CC = []                              # Pool rejects accum opcodes
ACT_CC = CC_TS                           # Sign counts on ACT

# accum column layout in `acc` [P, NFUNC, NKB]:
#   row 0:            sum vt4 (from the vt build stt)
#   rows 1..T0-1:     stt bins
#   next len(RS_TS):  RS
#   next len(CC_TS):  CC
_VT_ROW = 0
_STT_ROW = {b: b for b in STT_BINS}
_RS_ROW = {t: T0 + i for i, t in enumerate(RS_TS)}
_CC_ROW = {s: T0 + len(RS_TS) + i for i, s in enumerate(CC_TS)}
NFUNC = T0 + len(RS_TS) + len(CC_TS)     # 24

_CACHE = {}


_GROUP_STARTS = [0, 8]
while _GROUP_STARTS[-1] + G < CHUNKS:
    _GROUP_STARTS.append(_GROUP_STARTS[-1] + G)
_GROUP_BOUNDS = _GROUP_STARTS + [CHUNKS]
NGROUPS = len(_GROUP_STARTS)


def _slices_of_group(g):
    return range(_GROUP_BOUNDS[g], _GROUP_BOUNDS[g + 1])


def _kbs_of_group(g):
    return sorted({k // HB for k in _slices_of_group(g)})


def _slices_of_kb(kb):
    return range(kb * HB, min((kb + 1) * HB, CHUNKS))


def _build_program():
    from contextlib import ExitStack
    import concourse.bass as bass
    import concourse.tile as tile
    from concourse import bacc, mybir

    f32 = mybir.dt.float32
    f16 = mybir.dt.float16
    bf16 = mybir.dt.bfloat16
    ALU = mybir.AluOpType
    ACTF = mybir.ActivationFunctionType

    nc = bacc.Bacc("TRN2", target_bir_lowering=False, debug=False,
                   num_devices=N_CORES)

    lg = nc.dram_tensor("lg", [P, NF], bf16, kind="ExternalInput").ap()
    le = nc.dram_tensor("le", [P, NF], bf16, kind="ExternalInput").ap()
    w1 = nc.dram_tensor("w1", [P, G * SLOTS * G], bf16,
                        kind="ExternalInput").ap()
    w2 = nc.dram_tensor("w2", [G * SLOTS, G * P], f32,
                        kind="ExternalInput").ap()
    hacc = nc.dram_tensor("hacc", [P, NFUNC * NKB], f32,
                          kind="ExternalOutput").ap()

    SR = G * SLOTS           # 96 packed S rows per group

    with tile.TileContext(nc) as tc, ExitStack() as ctx:
        const_pool = ctx.enter_context(tc.tile_pool(name="const", bufs=1))
        lt_pool = ctx.enter_context(tc.tile_pool(name="lt", bufs=3))
        le_pool = ctx.enter_context(tc.tile_pool(name="le", bufs=2))
        et_pool = ctx.enter_context(tc.tile_pool(name="et", bufs=3))
        cf_pool = ctx.enter_context(tc.tile_pool(name="cf", bufs=2))
        vt_pool = ctx.enter_context(tc.tile_pool(name="vt", bufs=2))
        y_pool = ctx.enter_context(tc.tile_pool(name="y", bufs=1))
        bih_pool = ctx.enter_context(tc.tile_pool(name="bih", bufs=2))
        zp_pool = ctx.enter_context(tc.tile_pool(name="zp", bufs=2))
        td_pool = ctx.enter_context(tc.tile_pool(name="td", bufs=2))
        ta_pool = ctx.enter_context(tc.tile_pool(name="ta", bufs=2))
        rp_pool = ctx.enter_context(tc.tile_pool(name="rp", bufs=2))
        ps_s = ctx.enter_context(
            tc.tile_pool(name="ps_s", bufs=2, space=bass.MemorySpace.PSUM))
        ps_rb = ctx.enter_context(
            tc.tile_pool(name="ps_rb", bufs=1, space=bass.MemorySpace.PSUM))

        w1_sb = const_pool.tile([P, G * SR], bf16)
        nc.sync.dma_start(w1_sb[:], w1)
        w2_sb = const_pool.tile([SR, G * P], f32)
        nc.sync.dma_start(w2_sb[:], w2)
        acc = const_pool.tile([P, NFUNC * NKB], f32)
        sbias = const_pool.tile([P, max(1, len(ACT_CC))], f32)
        for i, s in enumerate(ACT_CC):
            nc.gpsimd.memset(sbias[:, i:i + 1], -float(s))
        rbias = const_pool.tile([P, len(RS_TS)], f32)
        for i, t in enumerate(RS_TS):
            nc.gpsimd.memset(rbias[:, i:i + 1], -float(t))

        lts = {}
        les = {}
        ets = {}

        for g in range(NGROUPS):
            ks = list(_slices_of_group(g))
            kbs = _kbs_of_group(g)
            for kb in kbs:
                if kb in ets:
                    continue
                fdb = len(list(_slices_of_kb(kb))) * FD
                off = kb * HB * FD
                ltb = lt_pool.tile([P, fdb], bf16, tag="lt")
                nc.sync.dma_start(ltb[:], lg[:, off:off + fdb])
                leb = le_pool.tile([P, fdb], bf16, tag="le")
                nc.sync.dma_start(leb[:], le[:, off:off + fdb])
                etb = et_pool.tile([P, fdb], bf16, tag="et")
                nc.scalar.activation(etb[:], ltb[:], ACTF.Exp)
                lts[kb] = ltb
                les[kb] = leb
                ets[kb] = etb

            srows = SLOTS * len(ks)
            spack = ps_s.tile([srows, FD], f32, tag="spack")
            for jg, k in enumerate(ks):
                kb, j = k // HB, k % HB
                etsl = ets[kb][:, j * FD:(j + 1) * FD]
                nc.tensor.matmul(
                    spack[:],
                    w1_sb[:, jg * SR:jg * SR + srows],
                    etsl,
                    start=(jg == 0), stop=(jg == len(ks) - 1))
            rpk = rp_pool.tile([srows, FD], f32, tag="rpk")
            nc.vector.reciprocal_approx_fast(rpk[:], spack[:])

            for kb in kbs:
                ksl = [k for k in _slices_of_kb(kb) if k in ks]
                assert len(ksl) == len(list(_slices_of_kb(kb))), \
                    "group/big-chunk misalignment"
                fdb = len(ksl) * FD
                etb = ets[kb]
                cfb = cf_pool.tile([P, fdb], f16, tag="cf")
                # rb batches of RBW chunks -> one wide cf4 multiply each
                for r0 in range(0, len(ksl), RBW):
                    rk = ksl[r0:r0 + RBW]
                    rbw = ps_rb.tile([P, len(rk) * FD], f32, tag="rb")
                    for q, k in enumerate(rk):
                        jg = k - _GROUP_BOUNDS[g]
                        nc.tensor.matmul(
                            rbw[:, q * FD:(q + 1) * FD],
                            w2_sb[:srows, jg * P:(jg + 1) * P],
                            rpk[:],
                            start=True, stop=True)
                    j0 = rk[0] % HB
                    nc.vector.tensor_mul(
                        cfb[:, j0 * FD:(j0 + len(rk)) * FD],
                        etb[:, j0 * FD:(j0 + len(rk)) * FD], rbw[:])

                leb = les.pop(kb)
                lts.pop(kb)
                ets.pop(kb)
                y16 = y_pool.tile([P, fdb], f16, tag="y16")
                nc.vector.tensor_scalar(
                    y16[:], cfb[:], 60.0, MAGIC16, op0=ALU.mult, op1=ALU.add)
                bih = bih_pool.tile([P, fdb], bf16, tag="bih")
                nc.vector.tensor_scalar(
                    bih[:], y16[:], -MAGIC16, 0.5, op0=ALU.add, op1=ALU.max)
                vtb = vt_pool.tile([P, fdb], f16, tag="vt")
                nc.vector.scalar_tensor_tensor(
                    vtb[:], leb[:], 1.0, cfb[:],
                    op0=ALU.mult, op1=ALU.subtract,
                    accum_out=acc[:, _VT_ROW * NKB + kb:_VT_ROW * NKB + kb + 1])
                zpb = zp_pool.tile([P, fdb], f16, tag="zp")
                nc.vector.tensor_add(zpb[:], bih[:], vtb[:])

                # low bins: direct masked D sums on DVE (stt, 1x)
                trd = td_pool.tile([P, fdb], f16, tag="trd")
                for b in STT_BINS:
                    col = _STT_ROW[b] * NKB + kb
                    nc.vector.scalar_tensor_tensor(
                        trd[:], bih[:], float(b) + 0.5, vtb[:],
                        op0=ALU.is_equal, op1=ALU.mult,
                        accum_out=acc[:, col:col + 1])
                # high bins: composite functionals on ACT + GpSimd
                tra = ta_pool.tile([P, fdb], f16, tag="tra")
                for i, t in enumerate(RS_TS):
                    col = _RS_ROW[t] * NKB + kb
                    nc.scalar.activation(
                        tra[:], zpb[:], ACTF.Relu,
                        bias=rbias[:, i:i + 1],
                        accum_out=acc[:, col:col + 1])
                for i, s in enumerate(ACT_CC):
                    col = _CC_ROW[s] * NKB + kb
                    nc.scalar.activation(
                        tra[:], zpb[:], ACTF.Sign,
                        bias=sbias[:, i:i + 1],
                        accum_out=acc[:, col:col + 1])

        nc.sync.dma_start(hacc, acc[:])

    nc.compile()
    return nc


def _get_program():
    if "nc" not in _CACHE:
        _CACHE["nc"] = _build_program()
    return _CACHE["nc"]


def _host_constants():
    import ml_dtypes
    SR = G * SLOTS
    w1 = np.zeros((P, G * SR), np.float32)
    w2 = np.zeros((SR, G * P), np.float32)
    for jg in range(G):
        for s in range(SLOTS):
            for c in range(C):
                p = s * C + c
                w1[p, jg * SR + SLOTS * jg + s] = 1.0
                w2[SLOTS * jg + s, jg * P + p] = 0.25
    return w1.astype(ml_dtypes.bfloat16), w2


def _decode(hsum, ncores=N_CORES):
    """hsum: [P, NFUNC*NKB] f64 (summed over cores) -> D_cb [19, 15]."""
    h = hsum.reshape(P, NFUNC, NKB)
    NT = ncores * NF
    sum_v4 = h[:, _VT_ROW, :].sum(axis=1)

    RS = np.zeros((P, NB + 1))
    CCm = np.zeros((P, NB + 1))
    for t in RS_TS:
        RS[:, t] = h[:, _RS_ROW[t], :].sum(axis=1)
    for s in CC_TS:
        val = h[:, _CC_ROW[s], :].sum(axis=1)
        if s in GPS_CC:
            CCm[:, s] = val
        else:
            CCm[:, s] = (val + NT) / 2.0

    D = np.zeros((P, NB))
    for b in STT_BINS:
        D[:, b] = 4.0 * h[:, _STT_ROW[b], :].sum(axis=1)
    for t in RS_TS:
        A = RS[:, t] - RS[:, t + 1] - CCm[:, t + 1]
        Cn = CCm[:, t] - CCm[:, t + 1]
        D[:, t] = 4.0 * A - 2.0 * Cn
    D[:, 0] = 4.0 * sum_v4 - D[:, 1:].sum(axis=1)

    return D.reshape(SLOTS, C, NB).sum(axis=0)


def kernel(logits, labels, _trace=False):
    import ml_dtypes
    from concourse.bass_utils import run_bass_kernel_spmd

    logits = np.asarray(logits, dtype=np.float32)
    labels = np.asarray(labels)
    lt = np.moveaxis(logits, 1, 0).reshape(C, N)
    lf = labels.reshape(N).astype(np.int32)

    w1, w2 = _host_constants()
    cids = np.arange(C, dtype=np.int32)
    in_maps = []
    for i in range(N_CORES):
        sl = slice(i * NPC, (i + 1) * NPC)
        lgc = np.zeros((C, NPIX), np.float32)
        lgc[:, :NPC] = lt[:, sl]
        lgc = np.ascontiguousarray(
            lgc.reshape(C, SLOTS, NF).transpose(1, 0, 2).reshape(P, NF)
        ).astype(ml_dtypes.bfloat16)
        lbc = np.zeros((NPIX,), np.int32)
        lbc[:NPC] = lf[sl]
        lec = (lbc.reshape(SLOTS, 1, NF) == cids[None, :, None])
        lec = np.ascontiguousarray(
            (lec.reshape(P, NF).astype(np.float32) * 0.25).astype(ml_dtypes.bfloat16))
        in_maps.append({"lg": lgc, "le": lec, "w1": w1, "w2": w2})

    nc = _get_program()
    res = run_bass_kernel_spmd(nc, in_maps, list(range(N_CORES)),
                               trace=_trace)
    _CACHE["last_exec_ns"] = res.exec_time_ns

    hsum = np.zeros((P, NFUNC * NKB), np.float64)
    for r in res.results:
        hsum += r["hacc"].astype(np.float64)
    D_cb = _decode(hsum)

    # remove zero-logit padding (label 0, conf 1/19 -> bin 0)
    pad_total = NPAD * N_CORES
    r19 = np.float64(np.float32(1.0) / np.float32(19.0))
    D_cb[:, 0] -= pad_total * ((np.arange(C) == 0).astype(np.float64) - r19)

    sce = np.abs(D_cb).sum(axis=1).mean() / N
    return np.float32(sce)
